# revision 16
# baseline (speedup 1.0000x reference)
"""Block-sparse attention TRN2 kernel (8 NeuronCores, SPMD over batch*heads).

Contract: kernel(**inputs) takes FULL unsharded inputs
  query/key/value: (2, 16, 2048, 128) f32, block_mask: (16, 16) bool,
  block_size: 128
and returns the FULL (2, 16, 2048, 128) f32 output.

Math per (b, h): for each 128x128 block pair (i, j) with block_mask[i, j]:
  A_ij = softmax(Q_i K_j^T / sqrt(128)) (softmax per block row, no
  cross-block merge), O_i = sum_j A_ij V_j.

Device layout ([k, q] orientation so no on-chip transposes are needed):
  For key block j, scores for the active query blocks are packed into
  1024-col (2 PSUM bank) chunks: S^T = matmul(lhsT=KT[:, j], rhs=QT runs),
  fp32r. exp on ACT (PSUM f32 -> SBUF f16). Denominators = column sums via
  matmul(lhsT=ones[128,128]) written back IN PLACE over the consumed score
  chunk (replicated across partitions). reciprocal_approx_fast (DVE), then
  Ahat = E * r elementwise (split DVE/GPSIMD). O^T += V_j^T.T @ Ahat^T
  accumulates in PSUM over j; drained via ACT copy + DMA per bank.
  Q^T/K^T/V packing and the final O^T -> O transpose happen on the host.
"""

import math

import numpy as np

B, H, S, D = 2, 16, 2048, 128
BS = 128
NB = S // BS
N_CORES = 8
N_HEADS = B * H
HPC = N_HEADS // N_CORES  # heads per core
BANK = 512  # PSUM bank, f32 elements
CH = 1024  # chunk columns (2 banks)
SCALE = 1.0 / math.sqrt(float(D))
# Fraction of normalize-multiplies kept on DVE; the rest go to the
# otherwise-idle GPSIMD (measured ~2.3ns/elem there vs ~1ns on DVE).
DVE_MULT_SHARE = 0.40


def _plan(mask):
    """Mask-derived emission plan (shared by every head/core).

    jplans[j] = list of chunks, each a dict with:
      banks:  [(boff, fill, mm1s)] per 512-col bank; mm1s = (off, qoffs, w)
              with off absolute in the chunk, len(qoffs)==2 for a paired
              single-block matmul.
      spans:  [(start, width)] contiguous used column ranges of the chunk.
      pieces: [(qo, wp, op)] MM2 output pieces (op = chunk col), split at
              output-bank boundaries and first-touch flips.
    """
    mask = np.asarray(mask).astype(bool)
    assert mask.shape == (NB, NB)
    first_j = {
        i: min(j for j in range(NB) if mask[i, j])
        for i in range(NB)
        if mask[i].any()
    }
    cap = BANK // BS  # blocks per bank (4)
    jplans = []
    bank_counts = [0] * (S // BANK)
    for j in range(NB):
        act = [i for i in range(NB) if mask[i, j]]
        runs = []
        for i in act:
            if runs and runs[-1][0] + runs[-1][1] == i:
                runs[-1][1] += 1
            else:
                runs.append([i, 1])
        items = []
        for i0, ln in runs:
            while ln > cap:
                items.append((i0, cap))
                i0 += cap
                ln -= cap
            items.append((i0, ln))
        # FFD bin packing into 512-col sub-bins (matmuls may not straddle a
        # PSUM bank; fp32r wants >=256-wide pieces).
        bins = []
        for i0, ln in sorted(items, key=lambda x: -x[1]):
            for b in bins:
                if b[0] + ln <= cap:
                    b[0] += ln
                    b[1].append((i0, ln))
                    break
            else:
                bins.append([ln, [(i0, ln)]])
        bins.sort(key=lambda b: -b[0])  # fullest first -> merged spans
        chunks = []
        for c0 in range(0, len(bins), 2):
            pair = bins[c0 : c0 + 2]
            banks = []
            spans = []
            pieces = []
            for bi, (fill, bitems) in enumerate(pair):
                boff = bi * BANK
                longs = sorted([it for it in bitems if it[1] > 1])
                singles = sorted([it for it in bitems if it[1] == 1])
                placed = []
                off = boff
                for i0, ln in longs + singles:
                    placed.append((off, i0 * BS, ln * BS))
                    off += ln * BS
                used = off - boff
                mm1s = []
                for o, q, w in placed:
                    if w > BS:
                        mm1s.append((o, [q], w))
                sing = [(o, q) for o, q, w in placed if w == BS]
                for k in range(0, len(sing) - 1, 2):
                    mm1s.append((sing[k][0], [sing[k][1], sing[k + 1][1]], 2 * BS))
                if len(sing) % 2:
                    mm1s.append((sing[-1][0], [sing[-1][1]], BS))
                banks.append((boff, used, mm1s))
                if spans and spans[-1][0] + spans[-1][1] == boff:
                    spans[-1] = (spans[-1][0], spans[-1][1] + used)
                else:
                    spans.append((boff, used))
                for o, qoff, w in placed:
                    ib0 = qoff // BS
                    nblk = w // BS
                    blk = 0
                    while blk < nblk:
                        ib = ib0 + blk
                        ft = first_j[ib] == j
                        obank = (ib * BS) // BANK
                        end = blk + 1
                        while end < nblk:
                            ib2 = ib0 + end
                            if (first_j[ib2] == j) != ft or (
                                ib2 * BS
                            ) // BANK != obank:
                                break
                            end += 1
                        qo = ib * BS
                        wp = (end - blk) * BS
                        pieces.append((qo, wp, o + (qo - qoff)))
                        bank_counts[obank] += 1
                        blk = end
            spans = [s for s in spans if s[1] > 0]
            chunks.append({"banks": banks, "spans": spans, "pieces": pieces})
        jplans.append(chunks)
    empty_rows = [i for i in range(NB) if not mask[i].any()]
    return jplans, bank_counts, empty_rows


def _build(mask):
    import concourse.bass as bass
    import concourse.bacc as bacc
    import concourse.tile as tile
    from concourse import mybir

    f32 = mybir.dt.float32
    f32r = mybir.dt.float32r
    f16 = mybir.dt.float16
    AF = mybir.ActivationFunctionType

    jplans, bank_counts, empty_rows = _plan(mask)

    nc = bacc.Bacc(
        "TRN2",
        target_bir_lowering=False,
        debug=False,
        enable_asserts=False,
        num_devices=N_CORES,
    )
    qt_d = nc.dram_tensor("qt", (HPC, D, S), f32r, kind="ExternalInput").ap()
    kt_d = nc.dram_tensor("kt", (HPC, D, S), f32r, kind="ExternalInput").ap()
    v_d = nc.dram_tensor("v", (HPC, BS, NB * BS), f16, kind="ExternalInput").ap()
    ot_d = nc.dram_tensor("ot", (HPC, D, S), f32, kind="ExternalOutput").ap()

    # Deterministic DVE/GPSIMD round-robin for the normalize multiply.
    mult_sched = {"acc": 0.0}

    def pick_mult_engine():
        mult_sched["acc"] += DVE_MULT_SHARE
        if mult_sched["acc"] >= 1.0:
            mult_sched["acc"] -= 1.0
            return nc.vector
        return nc.gpsimd

    with tile.TileContext(nc) as tc:
        with (
            tc.tile_pool(name="heads", bufs=2) as heads,
            tc.tile_pool(name="const", bufs=1) as const,
            tc.tile_pool(name="e", bufs=4) as epool,
            tc.tile_pool(name="eh", bufs=4) as ehpool,
            tc.tile_pool(name="r", bufs=4) as rpool,
            tc.tile_pool(name="outp", bufs=4) as outpool,
            tc.tile_pool(name="ps_s", bufs=2, space="PSUM") as ps_s,
            tc.tile_pool(name="ps_o", bufs=1, space="PSUM") as ps_o,
        ):
            ones_t = const.tile([BS, BS], f16)
            nc.vector.memset(ones_t[:], 1.0)

            for h in range(HPC):
                qt_t = heads.tile([D, S], f32r, tag="qt")
                nc.sync.dma_start(out=qt_t[:], in_=qt_d[h])
                kt_t = heads.tile([D, S], f32r, tag="kt")
                nc.sync.dma_start(out=kt_t[:], in_=kt_d[h])
                v_t = heads.tile([BS, NB * BS], f16, tag="v")
                nc.sync.dma_start(out=v_t[:], in_=v_d[h])

                o_ps = ps_o.tile([D, S], f32)
                for i in empty_rows:
                    nc.vector.memset(o_ps[:, i * BS : (i + 1) * BS], 0.0)

                remaining = list(bank_counts)
                started = set()
                for j in range(NB):
                    kt_j = kt_t[:, j * BS : (j + 1) * BS]
                    v_j = v_t[:, j * BS : (j + 1) * BS]
                    chunks = jplans[j]
                    if not chunks:
                        continue
                    stiles = []
                    etiles = []
                    ehtiles = []
                    # Phase 1: all score matmuls for this j (stationary KT_j).
                    for ch in chunks:
                        s_ps = ps_s.tile([BS, CH], f32)
                        stiles.append(s_ps)
                        for boff, used, mm1s in ch["banks"]:
                            for idx, (off, qoffs, w) in enumerate(mm1s):
                                if len(qoffs) == 2:
                                    base = qt_t[:, qoffs[0] : qoffs[0] + BS]
                                    rhs = bass.AP(
                                        tensor=base.tensor,
                                        offset=base.offset,
                                        ap=[
                                            base.ap[0],
                                            [qoffs[1] - qoffs[0], 2],
                                            [1, BS],
                                        ],
                                    )
                                else:
                                    rhs = qt_t[:, qoffs[0] : qoffs[0] + w]
                                nc.tensor.matmul(
                                    s_ps[:, off : off + w],
                                    lhsT=kt_j,
                                    rhs=rhs,
                                    start=(idx == 0),
                                    stop=(idx == len(mm1s) - 1),
                                )
                    # Phase 2: exp (ACT), per contiguous span.
                    for s_ps, ch in zip(stiles, chunks):
                        e_t = epool.tile([BS, CH], f16)
                        etiles.append(e_t)
                        for st, wd in ch["spans"]:
                            nc.scalar.activation(
                                e_t[:, st : st + wd],
                                s_ps[:, st : st + wd],
                                AF.Exp,
                                scale=SCALE,
                            )
                    # Phase 3: denominators, written in place over the
                    # consumed score banks (stationary ones).
                    for s_ps, e_t, ch in zip(stiles, etiles, chunks):
                        for boff, used, _ in ch["banks"]:
                            nc.tensor.matmul(
                                s_ps[:, boff : boff + used],
                                lhsT=ones_t[:],
                                rhs=e_t[:, boff : boff + used],
                                start=True,
                                stop=True,
                            )
                    # Phase 4+5: reciprocal (DVE) + normalize multiply.
                    for s_ps, e_t, ch in zip(stiles, etiles, chunks):
                        r_t = rpool.tile([BS, CH], f32)
                        eh_t = ehpool.tile([BS, CH], f16)
                        ehtiles.append(eh_t)
                        for st, wd in ch["spans"]:
                            nc.vector.reciprocal_approx_fast(
                                r_t[:, st : st + wd], s_ps[:, st : st + wd]
                            )
                            eng = pick_mult_engine()
                            eng.tensor_tensor(
                                out=eh_t[:, st : st + wd],
                                in0=e_t[:, st : st + wd],
                                in1=r_t[:, st : st + wd],
                                op=mybir.AluOpType.mult,
                            )
                    # Phase 6: output accumulation (stationary V_j).
                    for eh_t, ch in zip(ehtiles, chunks):
                        for qo, wp, op in ch["pieces"]:
                            b = qo // BANK
                            first = b not in started
                            started.add(b)
                            remaining[b] -= 1
                            nc.tensor.matmul(
                                o_ps[:, qo : qo + wp],
                                lhsT=v_j,
                                rhs=eh_t[:, op : op + wp],
                                start=first,
                                stop=(remaining[b] == 0),
                            )
                for b in range(S // BANK):
                    o_sb = outpool.tile([D, BANK], f32, tag="osb")
                    nc.scalar.copy(o_sb[:], o_ps[:, b * BANK : (b + 1) * BANK])
                    nc.sync.dma_start(
                        out=ot_d[h, :, b * BANK : (b + 1) * BANK], in_=o_sb[:]
                    )

    nc.finalize()
    return nc


_CACHE = {}


def _get_program(mask):
    key = np.asarray(mask).astype(bool).tobytes()
    if key not in _CACHE:
        _CACHE[key] = _build(mask)
    return _CACHE[key]


def _shard_inputs(query, key, value):
    q = np.ascontiguousarray(query, dtype=np.float32).reshape(N_HEADS, S, D)
    k = np.ascontiguousarray(key, dtype=np.float32).reshape(N_HEADS, S, D)
    v = np.ascontiguousarray(value, dtype=np.float32).reshape(N_HEADS, S, D)
    qt = np.ascontiguousarray(q.transpose(0, 2, 1))  # (32, D, S)
    kt = np.ascontiguousarray(k.transpose(0, 2, 1))
    v16 = np.ascontiguousarray(
        v.reshape(N_HEADS, NB, BS, D).transpose(0, 2, 1, 3).astype(np.float16)
    ).reshape(N_HEADS, BS, NB * BS)
    in_maps = []
    for c in range(N_CORES):
        sl = slice(c * HPC, (c + 1) * HPC)
        in_maps.append(
            {
                "qt": np.ascontiguousarray(qt[sl]),
                "kt": np.ascontiguousarray(kt[sl]),
                "v": np.ascontiguousarray(v16[sl]),
            }
        )
    return in_maps


def _unshard_output(results):
    ot = np.concatenate([r["ot"] for r in results], axis=0)  # (32, D, S)
    out = ot.transpose(0, 2, 1).reshape(B, H, S, D)
    return np.ascontiguousarray(out, dtype=np.float32)


def kernel(query, key, value, block_mask, block_size, _trace=False):
    from concourse.bass_utils import run_bass_kernel_spmd

    assert int(block_size) == BS
    nc = _get_program(block_mask)
    in_maps = _shard_inputs(query, key, value)
    res = run_bass_kernel_spmd(nc, in_maps, core_ids=list(range(N_CORES)), trace=_trace)
    out = _unshard_output(res.results)
    if _trace:
        return out, res
    return out


# revision 18
# speedup vs baseline: 1.5192x; 1.5192x over previous
"""Block-sparse attention TRN2 kernel (8 NeuronCores, SPMD over batch*heads).

Contract: kernel(**inputs) takes FULL unsharded inputs
  query/key/value: (2, 16, 2048, 128) f32, block_mask: (16, 16) bool,
  block_size: 128
and returns the FULL (2, 16, 2048, 128) f32 output.

Math per (b, h): for each 128x128 block pair (i, j) with block_mask[i, j]:
  A_ij = softmax(Q_i K_j^T / sqrt(128)) (softmax per block row, no
  cross-block merge), O_i = sum_j A_ij V_j.

Device layout ([k, q] orientation so no on-chip transposes are needed):
  For key block j, scores for the active query blocks are packed into
  512-col (one PSUM bank) chunks: S^T = matmul(lhsT=KT[:, j], rhs=QT runs)
  in fp32r (full-rate fp32). exp on ACT (PSUM f32 -> SBUF f16).
  Denominators = column sums via matmul(lhsT=ones[128,128]), replicated
  across partitions in PSUM. reciprocal_approx_fast (DVE), then
  Ahat = E * r elementwise (2 of 3 on DVE, 1 of 3 on the otherwise-idle
  GPSIMD). O^T += V_j^T.T @ Ahat^T accumulates in PSUM over j; drained
  via ACT copy + DMA per bank. Q^T/K^T/V packing and the final
  O^T -> O transpose happen on the host.
"""

import math

import numpy as np

B, H, S, D = 2, 16, 2048, 128
BS = 128
NB = S // BS
N_CORES = 8
N_HEADS = B * H
HPC = N_HEADS // N_CORES  # heads per core
CH = 512  # chunk columns = one PSUM bank of f32
SCALE = 1.0 / math.sqrt(float(D))


def _plan(mask):
    """Mask-derived emission plan (shared by every head/core).

    jplans[j] = list of chunks (used, mm1s, pieces):
      mm1s   = (off_in_chunk, [qoff, ...], width); two qoffs means a paired
               single-block matmul via a 3-level access pattern.
      pieces = (q_out_col, width, off_in_chunk) MM2 pieces, split at output
               PSUM bank boundaries and first-touch flips.
    """
    mask = np.asarray(mask).astype(bool)
    assert mask.shape == (NB, NB)
    first_j = {
        i: min(j for j in range(NB) if mask[i, j])
        for i in range(NB)
        if mask[i].any()
    }
    cap = CH // BS  # blocks per chunk
    jplans = []
    bank_counts = [0] * (S // CH)
    for j in range(NB):
        act = [i for i in range(NB) if mask[i, j]]
        runs = []
        for i in act:
            if runs and runs[-1][0] + runs[-1][1] == i:
                runs[-1][1] += 1
            else:
                runs.append([i, 1])
        # Items of at most `cap` blocks; long runs shed whole-chunk pieces.
        items = []
        for i0, ln in runs:
            while ln > cap:
                items.append((i0, cap))
                i0 += cap
                ln -= cap
            items.append((i0, ln))
        # First-fit-decreasing bin packing into chunks so matmuls never
        # straddle a chunk boundary (fp32r needs >=256-wide pieces to run
        # at full rate; splits create narrow ones).
        bins = []  # list of [fill_blocks, items]
        for i0, ln in sorted(items, key=lambda x: -x[1]):
            for b in bins:
                if b[0] + ln <= cap:
                    b[0] += ln
                    b[1].append((i0, ln))
                    break
            else:
                bins.append([ln, [(i0, ln)]])
        chunks = []
        for fill, bitems in bins:
            # Lay long items first; singles last, ascending, so adjacent
            # singles can share one paired-AP matmul (256-wide, full rate).
            longs = sorted([it for it in bitems if it[1] > 1])
            singles = sorted([it for it in bitems if it[1] == 1])
            placed = []  # (off, qoff, w)
            off = 0
            for i0, ln in longs + singles:
                placed.append((off, i0 * BS, ln * BS))
                off += ln * BS
            used = off
            mm1s = []  # (off, [qoff, ...], w)
            for o, q, w in placed:
                if w > BS:
                    mm1s.append((o, [q], w))
            sing = [(o, q) for o, q, w in placed if w == BS]
            for k in range(0, len(sing) - 1, 2):
                mm1s.append((sing[k][0], [sing[k][1], sing[k + 1][1]], 2 * BS))
            if len(sing) % 2:
                mm1s.append((sing[-1][0], [sing[-1][1]], BS))
            # MM2 pieces per placed item, split at output-bank boundaries and
            # wherever first-touch status flips (a single matmul's bytes must
            # be uniformly overwrite or uniformly accumulate).
            pieces = []
            for o, qoff, w in placed:
                ib0 = qoff // BS
                nblk = w // BS
                blk = 0
                while blk < nblk:
                    ib = ib0 + blk
                    ft = first_j[ib] == j
                    bank = (ib * BS) // CH
                    end = blk + 1
                    while end < nblk:
                        ib2 = ib0 + end
                        if (first_j[ib2] == j) != ft or (ib2 * BS) // CH != bank:
                            break
                        end += 1
                    qo = ib * BS
                    wp = (end - blk) * BS
                    pieces.append((qo, wp, o + (qo - qoff)))
                    bank_counts[bank] += 1
                    blk = end
            chunks.append((used, mm1s, pieces))
        jplans.append(chunks)
    empty_rows = [i for i in range(NB) if not mask[i].any()]
    return jplans, bank_counts, empty_rows


def _build(mask):
    import concourse.bass as bass
    import concourse.bacc as bacc
    import concourse.tile as tile
    from concourse import mybir

    f32 = mybir.dt.float32
    f32r = mybir.dt.float32r
    f16 = mybir.dt.float16
    AF = mybir.ActivationFunctionType

    jplans, bank_counts, empty_rows = _plan(mask)

    nc = bacc.Bacc(
        "TRN2",
        target_bir_lowering=False,
        debug=False,
        enable_asserts=False,
        num_devices=N_CORES,
    )
    qt_d = nc.dram_tensor("qt", (HPC, D, S), f32r, kind="ExternalInput").ap()
    kt_d = nc.dram_tensor("kt", (HPC, D, S), f32r, kind="ExternalInput").ap()
    v_d = nc.dram_tensor("v", (HPC, BS, NB * BS), f16, kind="ExternalInput").ap()
    ot_d = nc.dram_tensor("ot", (HPC, D, S), f32, kind="ExternalOutput").ap()

    with tile.TileContext(nc) as tc:
        with (
            tc.tile_pool(name="heads", bufs=2) as heads,
            tc.tile_pool(name="const", bufs=1) as const,
            tc.tile_pool(name="e", bufs=6) as epool,
            tc.tile_pool(name="eh", bufs=6) as ehpool,
            tc.tile_pool(name="r", bufs=6) as rpool,
            tc.tile_pool(name="outp", bufs=4) as outpool,
            tc.tile_pool(name="ps_s", bufs=2, space="PSUM") as ps_s,
            tc.tile_pool(name="ps_d", bufs=2, space="PSUM") as ps_d,
            tc.tile_pool(name="ps_o", bufs=1, space="PSUM") as ps_o,
        ):
            ones_t = const.tile([BS, BS], f16)
            nc.vector.memset(ones_t[:], 1.0)

            for h in range(HPC):
                qt_t = heads.tile([D, S], f32r, tag="qt")
                nc.sync.dma_start(out=qt_t[:], in_=qt_d[h])
                kt_t = heads.tile([D, S], f32r, tag="kt")
                nc.sync.dma_start(out=kt_t[:], in_=kt_d[h])
                v_t = heads.tile([BS, NB * BS], f16, tag="v")
                nc.sync.dma_start(out=v_t[:], in_=v_d[h])

                o_ps = ps_o.tile([D, S], f32)
                for i in empty_rows:
                    nc.vector.memset(o_ps[:, i * BS : (i + 1) * BS], 0.0)

                remaining = list(bank_counts)
                started = set()
                chunk_no = 0
                for j in range(NB):
                    kt_j = kt_t[:, j * BS : (j + 1) * BS]
                    v_j = v_t[:, j * BS : (j + 1) * BS]
                    for used, mm1s, pieces in jplans[j]:
                        s_ps = ps_s.tile([BS, CH], f32)
                        for idx, (off, qoffs, w) in enumerate(mm1s):
                            if len(qoffs) == 2:
                                base = qt_t[:, qoffs[0] : qoffs[0] + BS]
                                rhs = bass.AP(
                                    tensor=base.tensor,
                                    offset=base.offset,
                                    ap=[
                                        base.ap[0],
                                        [qoffs[1] - qoffs[0], 2],
                                        [1, BS],
                                    ],
                                )
                            else:
                                rhs = qt_t[:, qoffs[0] : qoffs[0] + w]
                            nc.tensor.matmul(
                                s_ps[:, off : off + w],
                                lhsT=kt_j,
                                rhs=rhs,
                                start=(idx == 0),
                                stop=(idx == len(mm1s) - 1),
                            )
                        e_t = epool.tile([BS, CH], f16)
                        nc.scalar.activation(
                            e_t[:, :used], s_ps[:, :used], AF.Exp, scale=SCALE
                        )
                        d_ps = ps_d.tile([BS, CH], f32)
                        nc.tensor.matmul(
                            d_ps[:, :used],
                            lhsT=ones_t[:],
                            rhs=e_t[:, :used],
                            start=True,
                            stop=True,
                        )
                        r_t = rpool.tile([BS, CH], f32)
                        nc.vector.reciprocal_approx_fast(r_t[:, :used], d_ps[:, :used])
                        eh_t = ehpool.tile([BS, CH], f16)
                        # Round-robin a third of the normalize multiplies onto
                        # the otherwise-idle GPSIMD engine (measured ~1.15us
                        # there vs ~0.58us on DVE; DVE is the busiest engine).
                        mult_eng = nc.gpsimd if chunk_no % 3 == 2 else nc.vector
                        chunk_no += 1
                        mult_eng.tensor_tensor(
                            out=eh_t[:, :used],
                            in0=e_t[:, :used],
                            in1=r_t[:, :used],
                            op=mybir.AluOpType.mult,
                        )
                        for qo, wp, op in pieces:
                            b = qo // CH
                            first = b not in started
                            started.add(b)
                            remaining[b] -= 1
                            nc.tensor.matmul(
                                o_ps[:, qo : qo + wp],
                                lhsT=v_j,
                                rhs=eh_t[:, op : op + wp],
                                start=first,
                                stop=(remaining[b] == 0),
                            )
                for b in range(S // CH):
                    o_sb = outpool.tile([D, CH], f32, tag="osb")
                    nc.scalar.copy(o_sb[:], o_ps[:, b * CH : (b + 1) * CH])
                    nc.sync.dma_start(
                        out=ot_d[h, :, b * CH : (b + 1) * CH], in_=o_sb[:]
                    )

    nc.finalize()
    return nc


_CACHE = {}


def _get_program(mask):
    key = np.asarray(mask).astype(bool).tobytes()
    if key not in _CACHE:
        _CACHE[key] = _build(mask)
    return _CACHE[key]


def _shard_inputs(query, key, value):
    q = np.ascontiguousarray(query, dtype=np.float32).reshape(N_HEADS, S, D)
    k = np.ascontiguousarray(key, dtype=np.float32).reshape(N_HEADS, S, D)
    v = np.ascontiguousarray(value, dtype=np.float32).reshape(N_HEADS, S, D)
    qt = np.ascontiguousarray(q.transpose(0, 2, 1))  # (32, D, S)
    kt = np.ascontiguousarray(k.transpose(0, 2, 1))
    v16 = np.ascontiguousarray(
        v.reshape(N_HEADS, NB, BS, D).transpose(0, 2, 1, 3).astype(np.float16)
    ).reshape(N_HEADS, BS, NB * BS)
    in_maps = []
    for c in range(N_CORES):
        sl = slice(c * HPC, (c + 1) * HPC)
        in_maps.append(
            {
                "qt": np.ascontiguousarray(qt[sl]),
                "kt": np.ascontiguousarray(kt[sl]),
                "v": np.ascontiguousarray(v16[sl]),
            }
        )
    return in_maps


def _unshard_output(results):
    ot = np.concatenate([r["ot"] for r in results], axis=0)  # (32, D, S)
    out = ot.transpose(0, 2, 1).reshape(B, H, S, D)
    return np.ascontiguousarray(out, dtype=np.float32)


def kernel(query, key, value, block_mask, block_size, _trace=False):
    from concourse.bass_utils import run_bass_kernel_spmd

    assert int(block_size) == BS
    nc = _get_program(block_mask)
    in_maps = _shard_inputs(query, key, value)
    res = run_bass_kernel_spmd(nc, in_maps, core_ids=list(range(N_CORES)), trace=_trace)
    out = _unshard_output(res.results)
    if _trace:
        return out, res
    return out
